# revision 86
# baseline (speedup 1.0000x reference)
"""Trainium2 Bass kernel for nn_Attention_34840774705279 (sparse/deformable attention).

Math (matches reference.py):
  v   = x @ v_w.T + v_b            -> per-head maps [B*NH, H, W, HD]
  off = x @ off_w.T + off_b        -> off_w is structurally zero, so offsets are
                                      CONSTANT per (head, point); for this problem
                                      they are (+-p or ~1e-16) => integer shifts.
  w   = softmax_p(x @ aw_w.T + aw_b)
  out[i,j] = sum_p w_p[i,j] * v[i+dy_p, j+dx_p]   (zero outside the map)
  y   = out @ proj_w.T + proj_b

Sharding (8 cores, uniform SPMD program):
  core d -> batch b = d//2, row-half r0 = 64*(d%2). Each core computes ALL 8
  heads for its 64 output rows (8192 tokens) using a 4-row halo of v rows
  (host zero-pads x rows outside the image), so shifts up to +-4 never cross
  cores and no cross-core reduction is needed; the host just concatenates.

Device algorithm (per core), fp16 on-chip / f32 PSUM accumulation:
  A. v+logit projection, pixel-major: host supplies x TRANSPOSED [256, 9216]
     in fp16; per image row r: a[j, 0:288] = xT_row_r.T @ [v_w.T | aw_w.T]
     lands in one PSUM bank and is copied once into the merged VL tile
     [j, 9, 32, 72] (slot h<8: per-head v maps stored d-major so the phase-C
     weight broadcast is middle-dim; slot 8: attention logits), fp16.
  B. softmax over the 4 points, batched across all 8 heads (fp16 z/recip).
  C. sampling + weighting via weight-then-shift identity
        w .* (S_dx @ V_win) == S_dx @ ((S_-dx^T w) .* V_win):
     two-stage pipeline (weights one head ahead of accumulation).  Per
     head-half: 4 tiny matmuls land the shifted weights for all 4 points in
     one PSUM bank, one copy evacuates them (dx=0 heads skip this - the
     e tile is used unshifted); the 32-row d-major V window is multiplied by
     the weight broadcast over d at DVE fp16 2x rate (the last point on the
     otherwise-idle Pool engine, which may not touch PSUM on this HW); the
     0/1 column-shift matrices matmul-accumulate all 4 points in PSUM.
  D. output projection: PE transposes OUT rows back to channel-major into
     fp16 PSUM, DVE evacuates (fp16-2x), y^T = proj.T @ OUT^T accumulated
     over both 128-channel halves, Act evacuates to fp16 staging, DMA out
     every 2 groups.  The full-half pass is software-pipelined (next group's
     transposes precede this group's projection matmuls).
  Emission interleaves C-half0 into phase A's tail (half-group granularity)
  and D-half0 into C-half1; PSUM->SBUF evacuation is placed per-region on
  Act/DVE so the Tensor engine stays the only near-critical resource.
"""

import os
import sys
import math

import numpy as np

sys.path.insert(0, "/opt/trn_rl_repo")

P = 128
H = W = 128
NH, NP, HD = 8, 4, 32
DIM = 256
N_TOK = H * W
ROWS_OUT = 64          # output rows per core
HALO = 4
ROWS_V = ROWS_OUT + 2 * HALO   # 72 v-row slots per core
TOK_V = ROWS_V * W             # 9216
N_CORES = 8

_cache = {}


def _build_terms(off_b):
    """Per (h, p): list of (dx, dy, alpha) corner terms from the constant offsets.

    General for any constant offset (bilinear corners); for this problem each
    (h, p) yields exactly one term with alpha ~= 1."""
    ob = np.asarray(off_b, np.float64).reshape(NH, NP, 2)
    terms = [[[] for _ in range(NP)] for _ in range(NH)]
    for h in range(NH):
        for p in range(NP):
            fx, fy = ob[h, p, 0], ob[h, p, 1]
            x0 = math.floor(fx)
            y0 = math.floor(fy)
            wx1 = fx - x0
            wy1 = fy - y0
            for dxc, wx in ((x0, 1.0 - wx1), (x0 + 1, wx1)):
                if abs(wx) < 1e-9:
                    continue
                for dyc, wy in ((y0, 1.0 - wy1), (y0 + 1, wy1)):
                    if abs(wy) < 1e-9:
                        continue
                    if abs(dxc) >= W or abs(dyc) > HALO:
                        continue  # fully out of range / beyond halo
                    terms[h][p].append((int(dxc), int(dyc), float(wx * wy)))
    return terms


def _build_smats(terms):
    """Dedupe (dx, alpha) -> [128,128] shift matrices; rewrite terms to
    (s_fwd, s_bwd, dy): out += S_dx @ (V_window * (alpha*S_-dx^T E))."""
    key_to_idx = {}
    mats = []

    def smat(dx, alpha):
        key = (dx, round(alpha, 9))
        if key not in key_to_idx:
            m = np.zeros((P, P), np.float32)
            for j_out in range(W):
                j_in = j_out + dx
                if 0 <= j_in < W:
                    m[j_in, j_out] = alpha
            key_to_idx[key] = len(mats)
            mats.append(m)
        return key_to_idx[key]

    terms2 = [[[] for _ in range(NP)] for _ in range(NH)]
    for h in range(NH):
        for p in range(NP):
            for dx, dy, alpha in terms[h][p]:
                terms2[h][p].append(
                    (smat(dx, 1.0), smat(-dx, alpha), dy))
    return np.stack(mats, 0), terms2


def _np_reference(x, v_w, v_b, aw_w, aw_b, off_w, off_b, proj_w, proj_b, Hh, Ww):
    """Pure-numpy fallback mirroring reference.py (used only if off_w != 0,
    which cannot happen with this problem's setup_inputs)."""
    B, N, C = x.shape
    v = (x @ v_w.T + v_b).reshape(B, N, NH, HD).transpose(0, 2, 1, 3)
    v = v.reshape(B * NH, Hh, Ww, HD)
    mh, mw = np.meshgrid(np.arange(Hh, dtype=x.dtype), np.arange(Ww, dtype=x.dtype),
                         indexing="ij")
    ref = np.stack([mw, mh], -1).reshape(1, N, 1, 2)
    off = (x @ off_w.T + off_b).reshape(B, N, NH, NP, 2).transpose(0, 2, 1, 3, 4)
    off = off.reshape(B * NH, N, NP, 2)
    grid = ref + off
    w = (x @ aw_w.T + aw_b).reshape(B, N, NH, NP).transpose(0, 2, 1, 3)
    w = w.reshape(B * NH, N, NP)
    w = np.exp(w - w.max(-1, keepdims=True))
    w = w / w.sum(-1, keepdims=True)
    G = B * NH
    vf = v.reshape(G, Hh * Ww, HD)
    gx, gy = grid[..., 0], grid[..., 1]
    x0 = np.floor(gx); y0 = np.floor(gy)
    wx1 = gx - x0; wx0 = 1.0 - wx1
    wy1 = gy - y0; wy0 = 1.0 - wy1
    x0i = x0.astype(np.int64); y0i = y0.astype(np.int64)

    def gather(xi, yi):
        valid = (xi >= 0) & (xi < Ww) & (yi >= 0) & (yi < Hh)
        idx = (np.clip(yi, 0, Hh - 1) * Ww + np.clip(xi, 0, Ww - 1))
        g = np.take_along_axis(vf, idx.reshape(G, -1, 1), axis=1)
        return g.reshape(*xi.shape, HD) * valid[..., None]

    samp = ((wy0 * wx0)[..., None] * gather(x0i, y0i)
            + (wy0 * wx1)[..., None] * gather(x0i + 1, y0i)
            + (wy1 * wx0)[..., None] * gather(x0i, y0i + 1)
            + (wy1 * wx1)[..., None] * gather(x0i + 1, y0i + 1))
    out = np.einsum("gnpd,gnp->gnd", samp, w)
    out = out.reshape(B, NH, N, HD).transpose(0, 2, 1, 3).reshape(B, N, C)
    return (out @ proj_w.T + proj_b).astype(np.float32)


def _build_program(terms, n_smats, has_bias=True, has_pbias=True):
    import concourse.bass as bass
    import concourse.mybir as mybir
    import concourse.tile as tile
    from concourse import bacc

    dt = mybir.dt
    f32 = dt.float32
    f16 = dt.float16

    nc = bacc.Bacc("TRN2", target_bir_lowering=False, debug=False,
                   num_devices=N_CORES)

    NCH = 256 + NH * NP  # 288: v channels + aw logits per row matmul

    # ---- DRAM I/O ----
    xt_d = nc.dram_tensor("xt_dev", [DIM, TOK_V], f16, kind="ExternalInput")
    ones_d = nc.dram_tensor("ones_dev", [1, TOK_V], f16, kind="ExternalInput")
    wb_d = nc.dram_tensor("wb_cat", [2, P, NCH], f16, kind="ExternalInput")
    bb_d = nc.dram_tensor("bb_cat", [1, NCH], f16, kind="ExternalInput")
    s_d = nc.dram_tensor("s_mats", [n_smats, P, P], f16, kind="ExternalInput")
    pj_d = nc.dram_tensor("proj_t", [2, 2, P, P], f16, kind="ExternalInput")
    pb_d = nc.dram_tensor("projb_t", [2, P], f32, kind="ExternalInput")
    id_d = nc.dram_tensor("ident", [P, P], f16, kind="ExternalInput")
    y0_d = nc.dram_tensor("y0", [P, ROWS_OUT * W], f16, kind="ExternalOutput")
    y1_d = nc.dram_tensor("y1", [P, ROWS_OUT * W], f16, kind="ExternalOutput")
    y_outs = [y0_d, y1_d]

    NG = ROWS_V // 8           # 9 x-DMA groups of 8 rows
    NGO = ROWS_OUT // 4        # 16 output groups (4 rows each) for phase D

    with tile.TileContext(nc) as tc:
        with (
            tc.tile_pool(name="const", bufs=1) as cpool,
            tc.tile_pool(name="big", bufs=1) as bigpool,
        ):
            # ---- constants (only wb DMA'd up front; the rest after the
            #      first x groups so phase A starts ASAP) ----
            wb_sb = cpool.tile([P, 2, NCH], f16, tag="wb")
            nc.sync.dma_start(wb_sb[:], wb_d.rearrange("kc k f -> k kc f"))
            bb_sb = cpool.tile([1, NCH], f16, tag="bb")
            s_sb = cpool.tile([P, n_smats, P], f16, tag="smats")
            pj_sb = cpool.tile([P, 2, 2, P], f16, tag="proj")
            pb_sb = cpool.tile([P, 2], f32, tag="projb")
            id_sb = cpool.tile([P, P], f16, tag="ident")

            def load_consts():
                if has_bias:
                    nc.sync.dma_start(bb_sb[:], bb_d[:])
                nc.sync.dma_start(s_sb[:], s_d.rearrange("s k f -> k s f"))

            def load_consts2():
                nc.sync.dma_start(pj_sb[:], pj_d.rearrange("kc m k f -> k kc m f"))
                if has_pbias:
                    nc.sync.dma_start(pb_sb[:], pb_d.rearrange("m k -> k m"))
                nc.sync.dma_start(id_sb[:], id_d[:])

            # ---- persistent big tiles ----
            # v maps d-major: [j, head-slot, d, row]; slot 8 = logits
            vl_sb = bigpool.tile([P, NH + 1, HD, ROWS_V], f16, tag="V")
            v_sb = vl_sb[:, :NH]
            outs = [bigpool.tile([P, 32, 2, P], f16, tag="OUT", name="out0"),
                    bigpool.tile([P, 32, 2, P], f16, tag="OUT2", name="out1")]
            es = [bigpool.tile([P, NH * NP, 32], f16, tag="E", name="e0"),
                  bigpool.tile([P, NH * NP, 32], f16, tag="E2", name="e1")]

            abc_pools = (
                tc.tile_pool(name="stA", bufs=2),
                tc.tile_pool(name="psA", bufs=2, space="PSUM"),
                tc.tile_pool(name="psC", bufs=2, space="PSUM"),
                tc.tile_pool(name="wt", bufs=1),
                tc.tile_pool(name="stB", bufs=2),
            )
            stA = abc_pools[0].__enter__()
            psA = abc_pools[1].__enter__()
            psC = abc_pools[2].__enter__()
            wtpool = abc_pools[3].__enter__()
            stB = abc_pools[4].__enter__()

            a_evac_cnt = [0]

            a_hold = {}

            def phase_a(g, rows=None):
                """x rows 8g..8g+8: v-proj + logits, pixel-major."""
                tok0 = g * 1024
                if rows is None:
                    rows = range(8)
                if 0 in rows:
                    xt_g = [stA.tile([P, 1024], f16, tag=f"xt{kc}", bufs=3,
                                     name=f"xtg{kc}") for kc in range(2)]
                    for kc in range(2):
                        nc.sync.dma_start(
                            xt_g[kc][:],
                            xt_d[P * kc:P * kc + P, tok0:tok0 + 1024])
                    ones_g = None
                    if has_bias:
                        ones_g = stA.tile([1, 1024], f16, tag="ones",
                                          name="ones_g")
                        nc.sync.dma_start(ones_g[:],
                                          ones_d[:, tok0:tok0 + 1024])
                    a_hold[g] = (xt_g, ones_g)
                xt_g, ones_g = a_hold[g]
                for rl in rows:
                    rr = 8 * g + rl      # v-row slot
                    a_ps = psA.tile([P, 512], f32, tag="a_ps", bufs=4)
                    for kc in range(2):
                        nc.tensor.matmul(
                            a_ps[:, :NCH],
                            xt_g[kc][:, P * rl:P * rl + P],
                            wb_sb[:, kc, :], start=(kc == 0),
                            stop=(kc == 1 and not has_bias))
                    if has_bias:
                        nc.tensor.matmul(
                            a_ps[:, :NCH], ones_g[:, P * rl:P * rl + P],
                            bb_sb[:], start=False, stop=True)
                    # evac: a[j, (h d)] -> vl[j, h, d, rr].  GPSIMD cannot
                    # read PSUM on real HW.  Early groups: DVE is idle until
                    # phase C starts, so split 1:1; late groups (overlapping
                    # phase C, where DVE saturates): Act only.
                    k = a_evac_cnt[0]; a_evac_cnt[0] += 1
                    if g < 5:
                        eng = (nc.scalar.copy, nc.vector.tensor_copy)[k % 2]
                    else:
                        eng = nc.scalar.copy
                    eng(vl_sb[:, :, :, rr],
                        a_ps[:, :NCH].rearrange("j (h d) -> j h d", h=NH + 1))

            def phase_b(half):
                """exp + softmax over points, all heads, rows of `half`."""
                rr = 32 * half
                e_sb = es[half]
                nc.scalar.activation(
                    e_sb[:],
                    vl_sb[:, NH, :, HALO + rr:HALO + rr + 32],
                    mybir.ActivationFunctionType.Exp)
                z = stB.tile([P, NH, 32], f16, tag="z")
                zr = stB.tile([P, NH, 32], f16, tag="zr")
                ev = e_sb[:].rearrange("j (h p) i -> j h p i", p=NP)
                nc.vector.tensor_tensor(z[:], ev[:, :, 0, :], ev[:, :, 1, :],
                                        op=mybir.AluOpType.add)
                nc.vector.tensor_tensor(z[:], z[:], ev[:, :, 2, :],
                                        op=mybir.AluOpType.add)
                nc.vector.tensor_tensor(z[:], z[:], ev[:, :, 3, :],
                                        op=mybir.AluOpType.add)
                with nc.allow_low_precision(
                        reason="softmax denom in [0.2,20]; fp16 ok"):
                    nc.vector.reciprocal(zr[:], z[:])
                for p in range(NP):
                    nc.vector.tensor_tensor(ev[:, :, p, :], ev[:, :, p, :],
                                            zr[:], op=mybir.AluOpType.mult)

            # heads whose every term is an unshifted alpha=1 sample (dx=0):
            # the backward weight shift is a no-op and the forward shift is
            # the identity matrix
            def _is_trivial(h):
                return all(
                    len(terms[h][p]) == 1
                    and terms[h][p][0][0] == terms[h][p][0][1]
                    for p in range(NP))
            trivial = [_is_trivial(h) for h in range(NH)]

            def phase_c_weights(half, h):
                """stage 1: shifted weights for all points of head h -> fp16
                SBUF (one PSUM bank + one DVE copy).  Run one head ahead of
                phase_c_accum so the Pool weighting op has lead time."""
                e_sb = es[half]
                if trivial[h]:
                    # weights are used unshifted -> e_sb slice, no matmuls
                    return e_sb[:, NP * h:NP * h + NP, :], NP
                ep_ps = psC.tile([P, NP, 32], f32, tag="ep", bufs=2)
                pi = 0
                for p in range(NP):
                    for (s_fwd, s_bwd, dy) in terms[h][p]:
                        nc.tensor.matmul(
                            ep_ps[:, pi, :], s_sb[:, s_bwd, :],
                            e_sb[:, NP * h + p, :], start=True, stop=True)
                        pi += 1
                ep = wtpool.tile([P, NP, 32], f16, tag="ep_sb",
                                 bufs=2, name="ep")
                # half0 overlaps phase A (Act saturated) -> DVE; half1
                # overlaps phase D (Act has slack) -> Act
                (nc.vector.tensor_copy if half == 0 else nc.scalar.copy)(
                    ep[:, :pi, :], ep_ps[:, :pi, :])
                return ep, pi

            def phase_c_accum(half, h, ep):
                """stage 2: weight (DVE/Pool fp16) + shift-accumulate (PE) +
                evacuate (Act)."""
                rr = 32 * half
                mh, hl = h // 4, h % 4
                o_ps = [psC.tile([P, 16, 32], f32, tag="oacc0",
                                 name="oacc0", bufs=1),
                        psC.tile([P, 16, 32], f32, tag="oacc1",
                                 name="oacc1", bufs=1)]
                n_terms = sum(len(terms[h][p]) for p in range(NP))
                m_ts = []
                t_seen = 0
                for p in range(NP):
                    for (s_fwd, s_bwd, dy) in terms[h][p]:
                        # last term's weighting runs on the otherwise-idle
                        # Pool engine (SBUF->SBUF is legal there); it gets a
                        # full head-cycle of lead time.
                        on_pool = (t_seen == n_terms - 1) and n_terms > 1
                        m_t = wtpool.tile(
                            [P, HD, 32], f16,
                            tag=("wtp" if on_pool else f"wt{t_seen % 2}"),
                            bufs=2 if on_pool else 3,
                            name=("mtp" if on_pool else f"mt{t_seen % 2}"))
                        slot0 = rr + dy + HALO
                        (nc.gpsimd.tensor_tensor if on_pool
                         else nc.vector.tensor_tensor)(
                            m_t[:], v_sb[:, h, :, slot0:slot0 + 32],
                            ep[:, t_seen, :].unsqueeze(1)
                            .broadcast_to([P, HD, 32]),
                            op=mybir.AluOpType.mult)
                        m_ts.append(m_t)
                        t_seen += 1
                t_seen = 0
                for p in range(NP):
                    for (s_fwd, s_bwd, dy) in terms[h][p]:
                        m_t = m_ts[t_seen]
                        for ch in range(2):
                            nc.tensor.matmul(
                                o_ps[ch][:].rearrange("j d i -> j (d i)"),
                                s_sb[:, s_fwd, :],
                                m_t[:, 16 * ch:16 * ch + 16, :]
                                .rearrange("j d i -> j (d i)"),
                                start=(t_seen == 0),
                                stop=(t_seen == n_terms - 1))
                        t_seen += 1
                # evac o[j, d, i] -> outs[j, i, mh, hl*HD + d].  GPSIMD
                # cannot read PSUM; in half1 (where DVE has slack) one of the
                # two halves goes to DVE.
                for ch in range(2):
                    dst = (outs[half][:, :, mh,
                                      HD * hl + 16 * ch:HD * hl + 16 * ch + 16]
                           .rearrange("j i d -> j d i"))
                    eng = (nc.vector.tensor_copy
                           if ch == 1 else nc.scalar.copy)
                    eng(dst, o_ps[ch][:])

            c_pending = [None]

            def phase_c(half, heads=None):
                """two-stage pipeline: weights for head h+1 overlap accum of
                head h (also across the half boundary)."""
                for h in (range(NH) if heads is None else heads):
                    if c_pending[0] is not None:
                        ph, phalf, pep = c_pending[0]
                        ep, _ = phase_c_weights(half, h)
                        c_pending[0] = (h, half, ep)
                        phase_c_accum(phalf, ph, pep)
                    else:
                        ep, _ = phase_c_weights(half, h)
                        c_pending[0] = (h, half, ep)

            def phase_c_flush():
                if c_pending[0] is not None:
                    ph, phalf, pep = c_pending[0]
                    c_pending[0] = None
                    phase_c_accum(phalf, ph, pep)

            ysb_hold = {}

            def phase_d_tr(halfd, g):
                """transposes + DVE evac for one group -> ot_sb tiles."""
                i0 = 4 * g
                ot_sb = []
                for m in range(2):
                    ot_ps = psA.tile([P, 4, P], f16, tag="a_ps",
                                     name=f"ot{m}", bufs=4)
                    for c in range(4):
                        nc.tensor.transpose(
                            ot_ps[:, c, :],
                            outs[halfd][:, i0 - 32 * halfd + c, m, :],
                            id_sb[:])
                    t = stA.tile([P, 512], f16, tag=f"ot{m}", bufs=2)
                    nc.vector.tensor_copy(
                        t[:], ot_ps[:].rearrange("k c f -> k (c f)"))
                    ot_sb.append(t)
                return ot_sb

            def phase_d(halfd, gls=None):
                """output projection for row groups of half `halfd`.  The
                full-half call is software-pipelined: group g+1's transposes
                are emitted before group g's y-matmuls so the PE never waits
                behind the cross-engine ot evacuation."""
                gl_list = list(range(NGO // 2) if gls is None else gls)
                pipelined = len(gl_list) > 1
                pend = None
                for gl in gl_list + [None]:
                    if gl is not None:
                        g = halfd * (NGO // 2) + gl
                        ot_sb = phase_d_tr(halfd, g)
                        if pipelined:
                            pend, cur = pend, (g, ot_sb)
                            if pend is None:
                                pend = cur
                                continue
                            g, ot_sb = pend
                            pend = cur
                    elif pipelined and pend is not None:
                        g, ot_sb = pend
                    else:
                        break
                    for mc in range(2):
                        y_ps = psA.tile([P, 512], f32, tag="a_ps",
                                        name=f"yps{mc}", bufs=4)
                        for kc in range(2):
                            nc.tensor.matmul(y_ps[:], pj_sb[:, kc, mc, :],
                                             ot_sb[kc][:],
                                             start=(kc == 0), stop=(kc == 1))
                        # evac into double-width staging; DMA every 2 groups.
                        # The pair tile is allocated at the even group and
                        # HELD so the odd group writes the same buffer.
                        if g >= NGO - 2 or g % 2 == 0:
                            ysb = stA.tile([P, 2, 512], f16, tag=f"y{mc}",
                                           name=f"ysb{mc}", bufs=4)
                            ysb_hold[mc] = ysb
                        else:
                            ysb = ysb_hold[mc]
                        if has_pbias:
                            nc.scalar.activation(
                                ysb[:, g % 2, :], y_ps[:],
                                mybir.ActivationFunctionType.Identity,
                                bias=pb_sb[:, mc:mc + 1])
                        else:
                            (nc.vector.tensor_copy if g == NGO - 1
                             else nc.scalar.copy)(ysb[:, g % 2, :], y_ps[:])
                        if g >= NGO - 2:
                            # tail groups: flush singly to shorten the drain
                            nc.sync.dma_start(
                                y_outs[mc][:, 512 * g:512 * (g + 1)],
                                ysb[:, g % 2, :])
                        elif g % 2 == 1:
                            nc.sync.dma_start(
                                y_outs[mc][:, 512 * (g - 1):512 * (g + 1)],
                                ysb[:].rearrange("k a f -> k (a f)"))

            # ---- emission order: A(0..4), B0, C0 overlap A(5..8), B1,
            #      C1 interleaved with D0, then D1
            for g in range(0, 5):
                phase_a(g)
            phase_b(0)
            load_consts()
            ci = 0
            for g in range(5, NG):
                phase_a(g, rows=range(0, 4))
                if ci < NH:
                    phase_c(0, heads=[ci]); ci += 1
                phase_a(g, rows=range(4, 8))
                if g == 6:
                    load_consts2()
                if ci < NH:
                    phase_c(0, heads=[ci]); ci += 1
            while ci < NH - 1:
                phase_c(0, heads=[ci]); ci += 1
            phase_b(1)
            while ci < NH:
                phase_c(0, heads=[ci]); ci += 1
            for h in range(NH):
                phase_c(1, heads=[h])
                phase_d(0, gls=[h])
            phase_c_flush()
            phase_d(1)
            if os.environ.get("KERNEL_DEBUG", "0") == "1":
                vl_dbg = nc.dram_tensor("vl_dbg", [P, NH + 1, HD, ROWS_V],
                                        f16, kind="ExternalOutput")
                e_dbg = nc.dram_tensor("e_dbg", [2, P, NH * NP, 32], f16,
                                       kind="ExternalOutput")
                o_dbg = nc.dram_tensor("o_dbg", [2, P, 32, 2, P], f16,
                                       kind="ExternalOutput")
                nc.sync.dma_start(vl_dbg[:], vl_sb[:])
                for hf in range(2):
                    nc.sync.dma_start(e_dbg[hf], es[hf][:])
                    nc.sync.dma_start(o_dbg[hf], outs[hf][:])
            for pl in reversed(abc_pools):
                pl.__exit__(None, None, None)

    nc.compile()
    return nc


def kernel(x, v_w, v_b, aw_w, aw_b, off_w, off_b, proj_w, proj_b, H=128, W=128,
           **_unused):
    x = np.ascontiguousarray(np.asarray(x, np.float32))
    v_w = np.asarray(v_w, np.float32); v_b = np.asarray(v_b, np.float32)
    aw_w = np.asarray(aw_w, np.float32); aw_b = np.asarray(aw_b, np.float32)
    off_w = np.asarray(off_w, np.float32); off_b = np.asarray(off_b, np.float32)
    proj_w = np.asarray(proj_w, np.float32); proj_b = np.asarray(proj_b, np.float32)

    if np.any(off_w != 0.0) or int(H) != 128 or int(W) != 128:
        # data-dependent offsets or non-128 map: exact host fallback
        return _np_reference(x, v_w, v_b, aw_w, aw_b, off_w, off_b,
                             proj_w, proj_b, int(H), int(W))

    terms = _build_terms(off_b)
    s_mats, terms2 = _build_smats(terms)

    has_bias = bool(np.any(v_b) or np.any(aw_b))
    has_pbias = bool(np.any(proj_b))
    key = ("prog", s_mats.shape[0], has_bias, has_pbias,
           tuple(tuple(tuple(tl) for tl in th) for th in terms2))
    if key not in _cache:
        _cache[key] = _build_program(terms2, s_mats.shape[0], has_bias,
                                     has_pbias)
    nc = _cache[key]

    B = x.shape[0]
    # ---- host prep, shared across cores ----
    NCH = 256 + NH * NP
    wb_cat = np.empty((2, P, NCH), np.float16)
    for kc in range(2):
        wb_cat[kc, :, :256] = v_w[:, P * kc:P * (kc + 1)].T
        wb_cat[kc, :, 256:] = aw_w[:, P * kc:P * (kc + 1)].T
    bb_cat = np.concatenate([v_b, aw_b]).reshape(1, NCH).astype(np.float16)
    pj_t = np.empty((2, 2, P, P), np.float16)
    for kc in range(2):
        for mc in range(2):
            pj_t[kc, mc] = proj_w[P * mc:P * (mc + 1), P * kc:P * (kc + 1)].T
    pb_t = proj_b.reshape(2, P).astype(np.float32)
    ident = np.eye(P, dtype=np.float16)
    shared = dict(wb_cat=np.ascontiguousarray(wb_cat),
                  bb_cat=np.ascontiguousarray(bb_cat),
                  s_mats=np.ascontiguousarray(s_mats.astype(np.float16)),
                  proj_t=np.ascontiguousarray(pj_t),
                  projb_t=np.ascontiguousarray(pb_t),
                  ident=ident)

    xr = x.reshape(B, H, W, DIM)
    in_maps = []
    for d in range(N_CORES):
        b, half = d // 2, d % 2
        r0 = ROWS_OUT * half
        x_dev = np.zeros((ROWS_V, W, DIM), np.float32)
        ones = np.zeros((ROWS_V, W), np.float16)
        lo, hi = max(0, r0 - HALO), min(H, r0 + ROWS_OUT + HALO)
        x_dev[lo - (r0 - HALO):hi - (r0 - HALO)] = xr[b, lo:hi]
        ones[lo - (r0 - HALO):hi - (r0 - HALO)] = 1.0
        m = dict(shared)
        m["xt_dev"] = np.ascontiguousarray(
            x_dev.reshape(TOK_V, DIM).T.astype(np.float16))
        m["ones_dev"] = ones.reshape(1, TOK_V)
        in_maps.append(m)

    from concourse import bass_utils
    res = bass_utils.run_bass_kernel_spmd(
        nc, in_maps, core_ids=list(range(N_CORES)),
        trace=os.environ.get("KERNEL_TRACE", "0") == "1")
    kernel.last_results = res

    y = np.empty((B, N_TOK, DIM), np.float32)
    for d in range(N_CORES):
        b, half = d // 2, d % 2
        yd = np.concatenate([np.asarray(res.results[d]["y0"], np.float32),
                             np.asarray(res.results[d]["y1"], np.float32)], 0)
        y[b, ROWS_OUT * W * half:ROWS_OUT * W * (half + 1), :] = yd.T
    return y


# revision 102
# speedup vs baseline: 1.0174x; 1.0174x over previous
"""Trainium2 Bass kernel for nn_Attention_34840774705279 (sparse/deformable attention).

Math (matches reference.py):
  v   = x @ v_w.T + v_b            -> per-head maps [B*NH, H, W, HD]
  off = x @ off_w.T + off_b        -> off_w is structurally zero, so offsets are
                                      CONSTANT per (head, point); for this problem
                                      they are (+-p or ~1e-16) => integer shifts.
  w   = softmax_p(x @ aw_w.T + aw_b)
  out[i,j] = sum_p w_p[i,j] * v[i+dy_p, j+dx_p]   (zero outside the map)
  y   = out @ proj_w.T + proj_b

Sharding (8 cores, uniform SPMD program):
  core d -> batch b = d//2, row-half r0 = 64*(d%2). Each core computes ALL 8
  heads for its 64 output rows (8192 tokens) using a 4-row halo of v rows
  (host zero-pads x rows outside the image), so shifts up to +-4 never cross
  cores and no cross-core reduction is needed; the host just concatenates.

Device algorithm (per core), fp16 on-chip / f32 PSUM accumulation:
  A. v+logit projection, pixel-major: host supplies x TRANSPOSED [256, 9216]
     in fp16; per image row r: a[j, 0:288] = xT_row_r.T @ [v_w.T | aw_w.T]
     lands in one PSUM bank and is copied once into the merged VL tile
     [j, 9, 32, 72] (slot h<8: per-head v maps stored d-major so the phase-C
     weight broadcast is middle-dim; slot 8: attention logits), fp16.
  B. softmax over the 4 points, batched across all 8 heads (fp16 z/recip).
  C. sampling + weighting via weight-then-shift identity
        w .* (S_dx @ V_win) == S_dx @ ((S_-dx^T w) .* V_win):
     two-stage pipeline (weights one head ahead of accumulation).  Per
     head-half: 4 tiny matmuls land the shifted weights for all 4 points in
     one PSUM bank, one copy evacuates them (dx=0 heads skip this - the
     e tile is used unshifted); the 32-row d-major V window is multiplied by
     the weight broadcast over d at DVE fp16 2x rate (the last point on the
     otherwise-idle Pool engine, which may not touch PSUM on this HW); the
     0/1 column-shift matrices matmul-accumulate all 4 points in PSUM.
  D. output projection: PE transposes OUT rows back to channel-major into
     fp16 PSUM, DVE evacuates (fp16-2x), y^T = proj.T @ OUT^T accumulated
     over both 128-channel halves, Act evacuates to fp16 staging, DMA out
     every 2 groups.  The full-half pass is software-pipelined (next group's
     transposes precede this group's projection matmuls).
  Emission interleaves C-half0 into phase A's tail (half-group granularity)
  and D-half0 into C-half1; PSUM->SBUF evacuation is placed per-region on
  Act/DVE so the Tensor engine stays the only near-critical resource.
"""

import os
import sys
import math

import numpy as np

sys.path.insert(0, "/opt/trn_rl_repo")

P = 128
H = W = 128
NH, NP, HD = 8, 4, 32
DIM = 256
N_TOK = H * W
ROWS_OUT = 64          # output rows per core
HALO = 4
ROWS_V = ROWS_OUT + 2 * HALO   # 72 v-row slots per core
TOK_V = ROWS_V * W             # 9216
N_CORES = 8

_cache = {}


def _build_terms(off_b):
    """Per (h, p): list of (dx, dy, alpha) corner terms from the constant offsets.

    General for any constant offset (bilinear corners); for this problem each
    (h, p) yields exactly one term with alpha ~= 1."""
    ob = np.asarray(off_b, np.float64).reshape(NH, NP, 2)
    terms = [[[] for _ in range(NP)] for _ in range(NH)]
    for h in range(NH):
        for p in range(NP):
            fx, fy = ob[h, p, 0], ob[h, p, 1]
            x0 = math.floor(fx)
            y0 = math.floor(fy)
            wx1 = fx - x0
            wy1 = fy - y0
            for dxc, wx in ((x0, 1.0 - wx1), (x0 + 1, wx1)):
                if abs(wx) < 1e-9:
                    continue
                for dyc, wy in ((y0, 1.0 - wy1), (y0 + 1, wy1)):
                    if abs(wy) < 1e-9:
                        continue
                    if abs(dxc) >= W or abs(dyc) > HALO:
                        continue  # fully out of range / beyond halo
                    terms[h][p].append((int(dxc), int(dyc), float(wx * wy)))
    return terms


def _build_smats(terms):
    """Dedupe (dx, alpha) -> [128,128] shift matrices; rewrite terms to
    (s_fwd, s_bwd, dy): out += S_dx @ (V_window * (alpha*S_-dx^T E))."""
    key_to_idx = {}
    mats = []

    def smat(dx, alpha):
        key = (dx, round(alpha, 9))
        if key not in key_to_idx:
            m = np.zeros((P, P), np.float32)
            for j_out in range(W):
                j_in = j_out + dx
                if 0 <= j_in < W:
                    m[j_in, j_out] = alpha
            key_to_idx[key] = len(mats)
            mats.append(m)
        return key_to_idx[key]

    terms2 = [[[] for _ in range(NP)] for _ in range(NH)]
    for h in range(NH):
        for p in range(NP):
            for dx, dy, alpha in terms[h][p]:
                terms2[h][p].append(
                    (smat(dx, 1.0), smat(-dx, alpha), dy))
    return np.stack(mats, 0), terms2


def _np_reference(x, v_w, v_b, aw_w, aw_b, off_w, off_b, proj_w, proj_b, Hh, Ww):
    """Pure-numpy fallback mirroring reference.py (used only if off_w != 0,
    which cannot happen with this problem's setup_inputs)."""
    B, N, C = x.shape
    v = (x @ v_w.T + v_b).reshape(B, N, NH, HD).transpose(0, 2, 1, 3)
    v = v.reshape(B * NH, Hh, Ww, HD)
    mh, mw = np.meshgrid(np.arange(Hh, dtype=x.dtype), np.arange(Ww, dtype=x.dtype),
                         indexing="ij")
    ref = np.stack([mw, mh], -1).reshape(1, N, 1, 2)
    off = (x @ off_w.T + off_b).reshape(B, N, NH, NP, 2).transpose(0, 2, 1, 3, 4)
    off = off.reshape(B * NH, N, NP, 2)
    grid = ref + off
    w = (x @ aw_w.T + aw_b).reshape(B, N, NH, NP).transpose(0, 2, 1, 3)
    w = w.reshape(B * NH, N, NP)
    w = np.exp(w - w.max(-1, keepdims=True))
    w = w / w.sum(-1, keepdims=True)
    G = B * NH
    vf = v.reshape(G, Hh * Ww, HD)
    gx, gy = grid[..., 0], grid[..., 1]
    x0 = np.floor(gx); y0 = np.floor(gy)
    wx1 = gx - x0; wx0 = 1.0 - wx1
    wy1 = gy - y0; wy0 = 1.0 - wy1
    x0i = x0.astype(np.int64); y0i = y0.astype(np.int64)

    def gather(xi, yi):
        valid = (xi >= 0) & (xi < Ww) & (yi >= 0) & (yi < Hh)
        idx = (np.clip(yi, 0, Hh - 1) * Ww + np.clip(xi, 0, Ww - 1))
        g = np.take_along_axis(vf, idx.reshape(G, -1, 1), axis=1)
        return g.reshape(*xi.shape, HD) * valid[..., None]

    samp = ((wy0 * wx0)[..., None] * gather(x0i, y0i)
            + (wy0 * wx1)[..., None] * gather(x0i + 1, y0i)
            + (wy1 * wx0)[..., None] * gather(x0i, y0i + 1)
            + (wy1 * wx1)[..., None] * gather(x0i + 1, y0i + 1))
    out = np.einsum("gnpd,gnp->gnd", samp, w)
    out = out.reshape(B, NH, N, HD).transpose(0, 2, 1, 3).reshape(B, N, C)
    return (out @ proj_w.T + proj_b).astype(np.float32)


def _build_program(terms, n_smats, has_bias=True, has_pbias=True):
    import concourse.bass as bass
    import concourse.mybir as mybir
    import concourse.tile as tile
    from concourse import bacc

    dt = mybir.dt
    f32 = dt.float32
    f16 = dt.float16

    nc = bacc.Bacc("TRN2", target_bir_lowering=False, debug=False,
                   num_devices=N_CORES)

    NCH = 256 + NH * NP  # 288: v channels + aw logits per row matmul

    # ---- DRAM I/O ----
    xt_d = nc.dram_tensor("xt_dev", [DIM, TOK_V], f16, kind="ExternalInput")
    ones_d = nc.dram_tensor("ones_dev", [1, TOK_V], f16, kind="ExternalInput")
    wb_d = nc.dram_tensor("wb_cat", [2, P, NCH], f16, kind="ExternalInput")
    bb_d = nc.dram_tensor("bb_cat", [1, NCH], f16, kind="ExternalInput")
    s_d = nc.dram_tensor("s_mats", [n_smats, P, P], f16, kind="ExternalInput")
    pj_d = nc.dram_tensor("proj_t", [2, 2, P, P], f16, kind="ExternalInput")
    pb_d = nc.dram_tensor("projb_t", [2, P], f32, kind="ExternalInput")
    id_d = nc.dram_tensor("ident", [P, P], f16, kind="ExternalInput")
    y0_d = nc.dram_tensor("y0", [P, ROWS_OUT * W], f16, kind="ExternalOutput")
    y1_d = nc.dram_tensor("y1", [P, ROWS_OUT * W], f16, kind="ExternalOutput")
    y_outs = [y0_d, y1_d]

    NG = ROWS_V // 8           # 9 x-DMA groups of 8 rows
    NGO = ROWS_OUT // 4        # 16 output groups (4 rows each) for phase D

    with tile.TileContext(nc) as tc:
        with (
            tc.tile_pool(name="const", bufs=1) as cpool,
            tc.tile_pool(name="big", bufs=1) as bigpool,
        ):
            # ---- constants (only wb DMA'd up front; the rest after the
            #      first x groups so phase A starts ASAP) ----
            wb_sb = cpool.tile([P, 2, NCH], f16, tag="wb")
            nc.sync.dma_start(wb_sb[:], wb_d.rearrange("kc k f -> k kc f"))
            bb_sb = cpool.tile([1, NCH], f16, tag="bb")
            s_sb = cpool.tile([P, n_smats, P], f16, tag="smats")
            pj_sb = cpool.tile([P, 2, 2, P], f16, tag="proj")
            pb_sb = cpool.tile([P, 2], f32, tag="projb")
            id_sb = cpool.tile([P, P], f16, tag="ident")

            def load_consts():
                if has_bias:
                    nc.sync.dma_start(bb_sb[:], bb_d[:])
                nc.sync.dma_start(s_sb[:], s_d.rearrange("s k f -> k s f"))

            def load_consts2():
                nc.sync.dma_start(pj_sb[:], pj_d.rearrange("kc m k f -> k kc m f"))
                if has_pbias:
                    nc.sync.dma_start(pb_sb[:], pb_d.rearrange("m k -> k m"))
                nc.sync.dma_start(id_sb[:], id_d[:])

            # ---- persistent big tiles ----
            # v maps d-major: [j, head-slot, d, row]; slot 8 = logits
            vl_sb = bigpool.tile([P, NH + 1, HD, ROWS_V], f16, tag="V")
            v_sb = vl_sb[:, :NH]
            outs = [bigpool.tile([P, 32, 2, P], f16, tag="OUT", name="out0"),
                    bigpool.tile([P, 32, 2, P], f16, tag="OUT2", name="out1")]
            es = [bigpool.tile([P, NH * NP, 32], f16, tag="E", name="e0"),
                  bigpool.tile([P, NH * NP, 32], f16, tag="E2", name="e1")]

            abc_pools = (
                tc.tile_pool(name="stA", bufs=2),
                tc.tile_pool(name="psA", bufs=2, space="PSUM"),
                tc.tile_pool(name="psC", bufs=2, space="PSUM"),
                tc.tile_pool(name="wt", bufs=1),
                tc.tile_pool(name="stB", bufs=2),
            )
            stA = abc_pools[0].__enter__()
            psA = abc_pools[1].__enter__()
            psC = abc_pools[2].__enter__()
            wtpool = abc_pools[3].__enter__()
            stB = abc_pools[4].__enter__()

            a_evac_cnt = [0]

            a_hold = {}

            def phase_a(g, rows=None):
                """x rows 8g..8g+8: v-proj + logits, pixel-major."""
                tok0 = g * 1024
                if rows is None:
                    rows = range(8)
                if 0 in rows:
                    xt_g = [stA.tile([P, 1024], f16, tag=f"xt{kc}", bufs=3,
                                     name=f"xtg{kc}") for kc in range(2)]
                    for kc in range(2):
                        nc.sync.dma_start(
                            xt_g[kc][:],
                            xt_d[P * kc:P * kc + P, tok0:tok0 + 1024])
                    ones_g = None
                    if has_bias:
                        ones_g = stA.tile([1, 1024], f16, tag="ones",
                                          name="ones_g")
                        nc.sync.dma_start(ones_g[:],
                                          ones_d[:, tok0:tok0 + 1024])
                    a_hold[g] = (xt_g, ones_g)
                xt_g, ones_g = a_hold[g]
                for rl in rows:
                    rr = 8 * g + rl      # v-row slot
                    a_ps = psA.tile([P, 512], f32, tag="a_ps", bufs=4)
                    for kc in range(2):
                        nc.tensor.matmul(
                            a_ps[:, :NCH],
                            xt_g[kc][:, P * rl:P * rl + P],
                            wb_sb[:, kc, :], start=(kc == 0),
                            stop=(kc == 1 and not has_bias))
                    if has_bias:
                        nc.tensor.matmul(
                            a_ps[:, :NCH], ones_g[:, P * rl:P * rl + P],
                            bb_sb[:], start=False, stop=True)
                    # evac: a[j, (h d)] -> vl[j, h, d, rr].  GPSIMD cannot
                    # read PSUM on real HW.  Early groups: DVE is idle until
                    # phase C starts, so split 1:1; late groups (overlapping
                    # phase C, where DVE saturates): Act only.
                    k = a_evac_cnt[0]; a_evac_cnt[0] += 1
                    if g < 5:
                        eng = (nc.scalar.copy, nc.vector.tensor_copy)[k % 2]
                    else:
                        eng = nc.scalar.copy
                    eng(vl_sb[:, :, :, rr],
                        a_ps[:, :NCH].rearrange("j (h d) -> j h d", h=NH + 1))

            def phase_b_exp(half):
                rr = 32 * half
                e_sb = es[half]
                nc.scalar.activation(
                    e_sb[:],
                    vl_sb[:, NH, :, HALO + rr:HALO + rr + 32],
                    mybir.ActivationFunctionType.Exp)

            def phase_b(half, exp=True):
                """exp + softmax over points, all heads, rows of `half`."""
                e_sb = es[half]
                if exp:
                    phase_b_exp(half)
                z = stB.tile([P, NH, 32], f16, tag="z")
                zr = stB.tile([P, NH, 32], f16, tag="zr")
                ev = e_sb[:].rearrange("j (h p) i -> j h p i", p=NP)
                nc.vector.tensor_tensor(z[:], ev[:, :, 0, :], ev[:, :, 1, :],
                                        op=mybir.AluOpType.add)
                nc.vector.tensor_tensor(z[:], z[:], ev[:, :, 2, :],
                                        op=mybir.AluOpType.add)
                nc.vector.tensor_tensor(z[:], z[:], ev[:, :, 3, :],
                                        op=mybir.AluOpType.add)
                with nc.allow_low_precision(
                        reason="softmax denom in [0.2,20]; fp16 ok"):
                    nc.vector.reciprocal(zr[:], z[:])
                for p in range(NP):
                    nc.vector.tensor_tensor(ev[:, :, p, :], ev[:, :, p, :],
                                            zr[:], op=mybir.AluOpType.mult)

            # heads whose every term is an unshifted alpha=1 sample (dx=0):
            # the backward weight shift is a no-op and the forward shift is
            # the identity matrix
            def _is_trivial(h):
                return all(
                    len(terms[h][p]) == 1
                    and terms[h][p][0][0] == terms[h][p][0][1]
                    for p in range(NP))
            trivial = [_is_trivial(h) for h in range(NH)]

            def phase_c_weights(half, h):
                """stage 1: shifted weights for all points of head h -> fp16
                SBUF (one PSUM bank + one DVE copy).  Run one head ahead of
                phase_c_accum so the Pool weighting op has lead time."""
                e_sb = es[half]
                if trivial[h]:
                    # weights are used unshifted -> e_sb slice, no matmuls
                    return e_sb[:, NP * h:NP * h + NP, :], NP
                ep_ps = psC.tile([P, NP, 32], f32, tag="ep", bufs=2)
                pi = 0
                for p in range(NP):
                    for (s_fwd, s_bwd, dy) in terms[h][p]:
                        nc.tensor.matmul(
                            ep_ps[:, pi, :], s_sb[:, s_bwd, :],
                            e_sb[:, NP * h + p, :], start=True, stop=True)
                        pi += 1
                ep = wtpool.tile([P, NP, 32], f16, tag="ep_sb",
                                 bufs=2, name="ep")
                # half0 overlaps phase A (Act saturated) -> DVE; half1
                # overlaps phase D (Act has slack) -> Act
                (nc.vector.tensor_copy if half == 0 else nc.scalar.copy)(
                    ep[:, :pi, :], ep_ps[:, :pi, :])
                return ep, pi

            def phase_c_accum(half, h, ep):
                """stage 2: weight (DVE/Pool fp16) + shift-accumulate (PE) +
                evacuate (Act)."""
                rr = 32 * half
                mh, hl = h // 4, h % 4
                o_ps = [psC.tile([P, 16, 32], f32, tag="oacc0",
                                 name="oacc0", bufs=1),
                        psC.tile([P, 16, 32], f32, tag="oacc1",
                                 name="oacc1", bufs=1)]
                n_terms = sum(len(terms[h][p]) for p in range(NP))
                m_ts = []
                t_seen = 0
                for p in range(NP):
                    for (s_fwd, s_bwd, dy) in terms[h][p]:
                        # last term's weighting runs on the otherwise-idle
                        # Pool engine (SBUF->SBUF is legal there); it gets a
                        # full head-cycle of lead time.
                        on_pool = (t_seen == n_terms - 1) and n_terms > 1
                        m_t = wtpool.tile(
                            [P, HD, 32], f16,
                            tag=("wtp" if on_pool else f"wt{t_seen % 2}"),
                            bufs=2 if on_pool else 3,
                            name=("mtp" if on_pool else f"mt{t_seen % 2}"))
                        slot0 = rr + dy + HALO
                        (nc.gpsimd.tensor_tensor if on_pool
                         else nc.vector.tensor_tensor)(
                            m_t[:], v_sb[:, h, :, slot0:slot0 + 32],
                            ep[:, t_seen, :].unsqueeze(1)
                            .broadcast_to([P, HD, 32]),
                            op=mybir.AluOpType.mult)
                        m_ts.append(m_t)
                        t_seen += 1
                t_seen = 0
                for p in range(NP):
                    for (s_fwd, s_bwd, dy) in terms[h][p]:
                        m_t = m_ts[t_seen]
                        for ch in range(2):
                            nc.tensor.matmul(
                                o_ps[ch][:].rearrange("j d i -> j (d i)"),
                                s_sb[:, s_fwd, :],
                                m_t[:, 16 * ch:16 * ch + 16, :]
                                .rearrange("j d i -> j (d i)"),
                                start=(t_seen == 0),
                                stop=(t_seen == n_terms - 1))
                        t_seen += 1
                # evac o[j, d, i] -> outs[j, i, mh, hl*HD + d].  GPSIMD
                # cannot read PSUM; in half1 (where DVE has slack) one of the
                # two halves goes to DVE.
                for ch in range(2):
                    dst = (outs[half][:, :, mh,
                                      HD * hl + 16 * ch:HD * hl + 16 * ch + 16]
                           .rearrange("j i d -> j d i"))
                    eng = (nc.vector.tensor_copy
                           if ch == 1 else nc.scalar.copy)
                    eng(dst, o_ps[ch][:])

            c_pending = [None]

            def phase_c(half, heads=None):
                """two-stage pipeline: weights for head h+1 overlap accum of
                head h (also across the half boundary)."""
                for h in (range(NH) if heads is None else heads):
                    if c_pending[0] is not None:
                        ph, phalf, pep = c_pending[0]
                        ep, _ = phase_c_weights(half, h)
                        c_pending[0] = (h, half, ep)
                        phase_c_accum(phalf, ph, pep)
                    else:
                        ep, _ = phase_c_weights(half, h)
                        c_pending[0] = (h, half, ep)

            def phase_c_flush():
                if c_pending[0] is not None:
                    ph, phalf, pep = c_pending[0]
                    c_pending[0] = None
                    phase_c_accum(phalf, ph, pep)

            ysb_hold = {}

            def phase_d_tr(halfd, g):
                """transposes + DVE evac for one group -> ot_sb tiles."""
                i0 = 4 * g
                ot_sb = []
                for m in range(2):
                    ot_ps = psA.tile([P, 4, P], f16, tag="a_ps",
                                     name=f"ot{m}", bufs=4)
                    for c in range(4):
                        nc.tensor.transpose(
                            ot_ps[:, c, :],
                            outs[halfd][:, i0 - 32 * halfd + c, m, :],
                            id_sb[:])
                    t = stA.tile([P, 512], f16, tag=f"ot{m}", bufs=2)
                    nc.vector.tensor_copy(
                        t[:], ot_ps[:].rearrange("k c f -> k (c f)"))
                    ot_sb.append(t)
                return ot_sb

            def phase_d(halfd, gls=None):
                """output projection for row groups of half `halfd`.  The
                full-half call is software-pipelined: group g+1's transposes
                are emitted before group g's y-matmuls so the PE never waits
                behind the cross-engine ot evacuation."""
                gl_list = list(range(NGO // 2) if gls is None else gls)
                pipelined = len(gl_list) > 1
                pend = None
                for gl in gl_list + [None]:
                    if gl is not None:
                        g = halfd * (NGO // 2) + gl
                        ot_sb = phase_d_tr(halfd, g)
                        if pipelined:
                            pend, cur = pend, (g, ot_sb)
                            if pend is None:
                                pend = cur
                                continue
                            g, ot_sb = pend
                            pend = cur
                    elif pipelined and pend is not None:
                        g, ot_sb = pend
                    else:
                        break
                    for mc in range(2):
                        y_ps = psA.tile([P, 512], f32, tag="a_ps",
                                        name=f"yps{mc}", bufs=4)
                        for kc in range(2):
                            nc.tensor.matmul(y_ps[:], pj_sb[:, kc, mc, :],
                                             ot_sb[kc][:],
                                             start=(kc == 0), stop=(kc == 1))
                        # evac into double-width staging; DMA every 2 groups.
                        # The pair tile is allocated at the even group and
                        # HELD so the odd group writes the same buffer.
                        if g >= NGO - 2 or g % 2 == 0:
                            ysb = stA.tile([P, 2, 512], f16, tag=f"y{mc}",
                                           name=f"ysb{mc}", bufs=4)
                            ysb_hold[mc] = ysb
                        else:
                            ysb = ysb_hold[mc]
                        if has_pbias:
                            nc.scalar.activation(
                                ysb[:, g % 2, :], y_ps[:],
                                mybir.ActivationFunctionType.Identity,
                                bias=pb_sb[:, mc:mc + 1])
                        else:
                            (nc.vector.tensor_copy if g == NGO - 1
                             else nc.scalar.copy)(ysb[:, g % 2, :], y_ps[:])
                        if g >= NGO - 2:
                            # tail groups: flush singly to shorten the drain
                            nc.sync.dma_start(
                                y_outs[mc][:, 512 * g:512 * (g + 1)],
                                ysb[:, g % 2, :])
                        elif g % 2 == 1:
                            nc.sync.dma_start(
                                y_outs[mc][:, 512 * (g - 1):512 * (g + 1)],
                                ysb[:].rearrange("k a f -> k (a f)"))

            # ---- emission order: warmup, A(0..4), B0, C0 overlap
            #      A(5..8), B1, C1 interleaved with D0, then D1
            n_warm = int(os.environ.get("KERNEL_WARMUP", "4"))
            if n_warm:
                # dummy matmuls on a memset tile keep the PE busy from ~0.7us
                # so the p-state ramp completes before the first x tile lands
                wtile = cpool.tile([P, P], f16, tag="warm")
                nc.gpsimd.memset(wtile[:], 0)
                warm_ps = psA.tile([P, 512], f32, tag="a_ps", bufs=4,
                                   name="warm")
                for _ in range(n_warm):
                    nc.tensor.matmul(warm_ps[:, :64], wtile[:],
                                     wtile[:, :64], start=True, stop=True)
            for g in range(0, 5):
                phase_a(g)
            phase_b(0)
            load_consts()
            ci = 0
            for g in range(5, NG):
                phase_a(g, rows=range(0, 4))
                if ci < NH:
                    phase_c(0, heads=[ci]); ci += 1
                phase_a(g, rows=range(4, 8))
                if g == 6:
                    load_consts2()
                if g == NG - 1:
                    # half1 exp right behind group 8's evacs in the Act
                    # queue; the DVE softmax follows after the C0 tail
                    phase_b_exp(1)
                if ci < NH:
                    phase_c(0, heads=[ci]); ci += 1
            while ci < NH - 2:
                phase_c(0, heads=[ci]); ci += 1
            phase_b(1, exp=False)
            while ci < NH:
                phase_c(0, heads=[ci]); ci += 1
            for h in range(NH):
                phase_c(1, heads=[h])
                phase_d(0, gls=[h])
            phase_c_flush()
            phase_d(1)
            if os.environ.get("KERNEL_DEBUG", "0") == "1":
                vl_dbg = nc.dram_tensor("vl_dbg", [P, NH + 1, HD, ROWS_V],
                                        f16, kind="ExternalOutput")
                e_dbg = nc.dram_tensor("e_dbg", [2, P, NH * NP, 32], f16,
                                       kind="ExternalOutput")
                o_dbg = nc.dram_tensor("o_dbg", [2, P, 32, 2, P], f16,
                                       kind="ExternalOutput")
                nc.sync.dma_start(vl_dbg[:], vl_sb[:])
                for hf in range(2):
                    nc.sync.dma_start(e_dbg[hf], es[hf][:])
                    nc.sync.dma_start(o_dbg[hf], outs[hf][:])
            for pl in reversed(abc_pools):
                pl.__exit__(None, None, None)

    nc.compile()
    return nc


def kernel(x, v_w, v_b, aw_w, aw_b, off_w, off_b, proj_w, proj_b, H=128, W=128,
           **_unused):
    x = np.ascontiguousarray(np.asarray(x, np.float32))
    v_w = np.asarray(v_w, np.float32); v_b = np.asarray(v_b, np.float32)
    aw_w = np.asarray(aw_w, np.float32); aw_b = np.asarray(aw_b, np.float32)
    off_w = np.asarray(off_w, np.float32); off_b = np.asarray(off_b, np.float32)
    proj_w = np.asarray(proj_w, np.float32); proj_b = np.asarray(proj_b, np.float32)

    if np.any(off_w != 0.0) or int(H) != 128 or int(W) != 128:
        # data-dependent offsets or non-128 map: exact host fallback
        return _np_reference(x, v_w, v_b, aw_w, aw_b, off_w, off_b,
                             proj_w, proj_b, int(H), int(W))

    terms = _build_terms(off_b)
    s_mats, terms2 = _build_smats(terms)

    has_bias = bool(np.any(v_b) or np.any(aw_b))
    has_pbias = bool(np.any(proj_b))
    key = ("prog", s_mats.shape[0], has_bias, has_pbias,
           tuple(tuple(tuple(tl) for tl in th) for th in terms2))
    if key not in _cache:
        _cache[key] = _build_program(terms2, s_mats.shape[0], has_bias,
                                     has_pbias)
    nc = _cache[key]

    B = x.shape[0]
    # ---- host prep, shared across cores ----
    NCH = 256 + NH * NP
    wb_cat = np.empty((2, P, NCH), np.float16)
    for kc in range(2):
        wb_cat[kc, :, :256] = v_w[:, P * kc:P * (kc + 1)].T
        wb_cat[kc, :, 256:] = aw_w[:, P * kc:P * (kc + 1)].T
    bb_cat = np.concatenate([v_b, aw_b]).reshape(1, NCH).astype(np.float16)
    pj_t = np.empty((2, 2, P, P), np.float16)
    for kc in range(2):
        for mc in range(2):
            pj_t[kc, mc] = proj_w[P * mc:P * (mc + 1), P * kc:P * (kc + 1)].T
    pb_t = proj_b.reshape(2, P).astype(np.float32)
    ident = np.eye(P, dtype=np.float16)
    shared = dict(wb_cat=np.ascontiguousarray(wb_cat),
                  bb_cat=np.ascontiguousarray(bb_cat),
                  s_mats=np.ascontiguousarray(s_mats.astype(np.float16)),
                  proj_t=np.ascontiguousarray(pj_t),
                  projb_t=np.ascontiguousarray(pb_t),
                  ident=ident)

    xr = x.reshape(B, H, W, DIM)
    in_maps = []
    for d in range(N_CORES):
        b, half = d // 2, d % 2
        r0 = ROWS_OUT * half
        x_dev = np.zeros((ROWS_V, W, DIM), np.float32)
        ones = np.zeros((ROWS_V, W), np.float16)
        lo, hi = max(0, r0 - HALO), min(H, r0 + ROWS_OUT + HALO)
        x_dev[lo - (r0 - HALO):hi - (r0 - HALO)] = xr[b, lo:hi]
        ones[lo - (r0 - HALO):hi - (r0 - HALO)] = 1.0
        m = dict(shared)
        m["xt_dev"] = np.ascontiguousarray(
            x_dev.reshape(TOK_V, DIM).T.astype(np.float16))
        m["ones_dev"] = ones.reshape(1, TOK_V)
        in_maps.append(m)

    from concourse import bass_utils
    res = bass_utils.run_bass_kernel_spmd(
        nc, in_maps, core_ids=list(range(N_CORES)),
        trace=os.environ.get("KERNEL_TRACE", "0") == "1")
    kernel.last_results = res

    y = np.empty((B, N_TOK, DIM), np.float32)
    for d in range(N_CORES):
        b, half = d // 2, d % 2
        yd = np.concatenate([np.asarray(res.results[d]["y0"], np.float32),
                             np.asarray(res.results[d]["y1"], np.float32)], 0)
        y[b, ROWS_OUT * W * half:ROWS_OUT * W * (half + 1), :] = yd.T
    return y
